# revision 27
# baseline (speedup 1.0000x reference)
"""BitNet ternary linear layer on 8 Trainium2 NeuronCores.

out[b, o] = (sum_i w[o,i] * round(x[b,i]/act_scale)) * weight_scale * act_scale + bias[o]
  with w = unpack2bit(packed_weight) - 1   (codes c in {0..3} -> w in {-1..2})
  and  act_scale = max(absmax(x), 1e-5) / 127.

Strategy (tensor-parallel, column sharded over out_features):
 - The on-device 2bit->fp8 plane unpack is DVE-I/O-bound (the plane ops run
   at the DVE's 8B-read+8B-write per-lane-cycle ceiling, ~21.7us/core for
   the full 28 MiB expansion), while the weight DMA stream has ~8us of
   slack (7 MiB in ~13.2us measured).  So the host ships 1/8 of the
   out-features (o-chunk 7 of 8) PRE-decoded to 1-byte-per-code fp8 plane
   bytes, trading 3x extra DMA bytes for that slice against a 1/8 cut in
   DVE work: DMA ~18.3us vs DVE ~19.5us -- balanced.
 - Host layout per block q (2 j-tiles): [packed sub(2) x 3136 | predecoded
   sub(2) x k(4) x 448] = 9856 B/partition; partition p holds code-group
   j = jt*128+p.  Only the low byte of each packed int32 word is meaningful.
 - Device (per core, identical program):
   * quantize x on-chip: absmax (DVE partial + GPSIMD all-reduce) ->
     r=127/absmax -> xq = rne(x*r) via magic-number rounding on ACT,
     pre-scaled by g512 = 512*gamma into a bf16 stationary operand (ACT).
   * stream weight blocks; unpack 2-bit planes for chunks 0-6 with one
     fused DVE op per plane ((word >> 2k) & 0x03030303); chunk 7's operands
     come straight from the predecoded DMA bytes.  Bytes {0..3} are read
     as fp8e4 denormals with exact value c * 2^-9, so the PE computes
     g512*xq * c*2^-9 = gamma*xq*c directly (mixed-dtype matmul), 4 output
     chunks concurrently via PE column tiling.
   * the code-minus-one and bias are folded into PSUM with one extra K=2
     matmul per output chunk: rank-1 terms (-gamma*Sx[b])*1[o] + 1[b]*bias[o].
   * DMA issue order staggers the predecoded streams one block behind the
     packed streams so the DVE never waits on DMA and the PE's chunk-7
     matmuls never wait long on the predecoded bytes.
   * PSUM -> SBUF copies all run on ACT (it has slack; DVE is the binding
     engine), then DMA to HBM.
"""

import os
import sys

sys.path.insert(0, "/opt/trn_rl_repo")

import numpy as np

import concourse.bacc as bacc
import concourse.mybir as mybir
from concourse import bass_isa
from concourse import tile
from concourse.bass_utils import run_bass_kernel_spmd

AluOp = mybir.AluOpType
ActFn = mybir.ActivationFunctionType
dt = mybir.dt

O, I, B = 28672, 8192, 8
NCORES = 8
OS = O // NCORES          # 3584 out-features per core
J = I // 4                # 2048 packed byte-groups per out-feature
NJT = J // 128            # 16 j-tiles
JPB = 2                   # j-tiles per DMA block
NB = NJT // JPB           # 8 DMA blocks
MAGIC = 12582912.0        # 1.5 * 2^23: magic RNE round-to-integer constant

CH = 448                  # o-chunk size: 8 chunks, 2 per PE column group
NG = 4                    # PE column groups
NPK = 7                   # chunks with packed bytes on device (chunk 7 is
                          # fully predecoded on the host)
# PD6K: how many of chunk 6's four bit-planes are predecoded on the host
# (the k >= 4-PD6K planes).  Those DVE plane ops then cover chunks 0-5 only.
PD6K = int(os.environ.get("BITNET_PD6K", "1"))
PDRING = os.environ.get("BITNET_PDRING", "sp")  # "sp" | "act"
OW = NPK * CH             # 3136 packed o-width per sub
NPD = 4 + PD6K            # predecoded 448-wide plane slices per sub
PKB = JPB * OW            # 6272 packed bytes per partition per block
PDB = JPB * NPD * CH      # predecoded bytes per partition per block
BBLK = PKB + PDB          # bytes per partition per block
OWN = OW - CH             # 2688: narrowed o-width for predecoded-chunk6 planes

_cache = {}
LAST_RESULTS = None


def _build(repeat=1, mode="full", **_ignored):
    # mode: "full" = real kernel; "fullnq" = quant once (perf bisection);
    #       "dmaraw" = weight stream only; "planesonly" = DVE unpack on
    #       resident data; "peonly" = matmul stream on resident data;
    #       "quantonly"/"quantnog" = quant chain only.
    nc = bacc.Bacc("TRN2", target_bir_lowering=False, debug=False)

    pt = nc.dram_tensor("pt", [128, NB * BBLK], dt.uint8, kind="ExternalInput")
    xs = nc.dram_tensor("xs", [128, 512], dt.float32, kind="ExternalInput")
    ext = nc.dram_tensor("ext", [2, OS], dt.float32, kind="ExternalInput")
    ws = nc.dram_tensor("ws", [1, 1], dt.float32, kind="ExternalInput")
    idn = nc.dram_tensor("idn", [128, 128], dt.float32, kind="ExternalInput")
    out = nc.dram_tensor("out", [8, OS], dt.float32, kind="ExternalOutput")

    def plane_op(nc, dst, src_i32, k):
        # dst int32 slice <- (src >> 2k) & 0x03030303
        if k == 0:
            nc.vector.tensor_scalar(
                out=dst, in0=src_i32, scalar1=0x03030303, scalar2=None,
                op0=AluOp.bitwise_and,
            )
        else:
            nc.vector.tensor_scalar(
                out=dst, in0=src_i32, scalar1=2 * k, scalar2=0x03030303,
                op0=AluOp.logical_shift_right, op1=AluOp.bitwise_and,
            )

    def kwidth(k):
        # int32 words per sub decoded on-device for plane k
        return (OWN if k >= 4 - PD6K else OW) // 4

    def plane_full(nc, plpool, cb_t, k):
        # one fused op covering both subs of a block for plane k
        w = kwidth(k)
        pk = plpool.tile([128, JPB * w], dt.int32, tag=f"pk{k}",
                         name=f"pk{k}")
        if w == OW // 4:
            plane_op(nc, pk[:], cb_t[:].bitcast(dt.int32), k)
        else:
            src3 = cb_t[:].bitcast(dt.int32).rearrange(
                "p (s w) -> p s w", s=JPB
            )[:, :, 0:w]
            dst3 = pk[:].rearrange("p (s w) -> p s w", s=JPB)
            plane_op(nc, dst3, src3, k)
        return pk

    with tile.TileContext(nc) as tc:
        with (
            tc.tile_pool(name="io", bufs=2) as io,
            tc.tile_pool(name="wpool", bufs=4) as wpool,
            tc.tile_pool(name="pdpool", bufs=4) as pdpool,
            tc.tile_pool(name="plpool", bufs=3) as plpool,
            tc.tile_pool(name="opool", bufs=2) as opool,
            tc.tile_pool(name="ps", bufs=2, space="PSUM") as ps,
            tc.tile_pool(name="ps2", bufs=1, space="PSUM") as ps2,
        ):
            xs_t = io.tile([128, 512], dt.float32)
            nc.sync.dma_start(xs_t[:], xs[:])
            ext_t = io.tile([2, OS], dt.float32)
            nc.sync.dma_start(ext_t[:], ext[:])
            ws_t = io.tile([1, 1], dt.float32)
            nc.sync.dma_start(ws_t[:], ws[:])
            ws_b = io.tile([128, 1], dt.float32)
            nc.gpsimd.partition_broadcast(ws_b[:], ws_t[:])
            ones_t = io.tile([128, 1], dt.float32)
            nc.vector.memset(ones_t[:], 1.0)
            magic_t = io.tile([128, 1], dt.float32)
            nc.vector.memset(magic_t[:], MAGIC)
            nmagic_t = io.tile([128, 1], dt.float32)
            nc.vector.memset(nmagic_t[:], -MAGIC)
            idn_t = io.tile([128, 128], dt.float32)
            nc.sync.dma_start(idn_t[:], idn[:])
            ones_r = io.tile([1, 128], dt.float32)
            nc.vector.memset(ones_r[:], 1.0)

            def issue_dmas():
                # staggered order: p0a p0b p1 pd0 p2 pd1 ... p7 pd6 pd7
                cb = [None] * NB
                pd = [None] * NB

                def dma_p(q, split=False):
                    cb[q] = wpool.tile(
                        [128, PKB], dt.uint8, tag="cb", name=f"cb{q}"
                    )
                    base = q * BBLK
                    if split:
                        nc.sync.dma_start(cb[q][:, 0:OW], pt[:, base:base + OW])
                        nc.sync.dma_start(
                            cb[q][:, OW:PKB], pt[:, base + OW:base + PKB]
                        )
                    else:
                        nc.sync.dma_start(cb[q][:], pt[:, base:base + PKB])

                pd_eng = nc.scalar if PDRING == "act" else nc.sync

                def dma_pd(q):
                    pd[q] = pdpool.tile(
                        [128, PDB], dt.uint8, tag="pd", name=f"pd{q}"
                    )
                    base = q * BBLK + PKB
                    pd_eng.dma_start(pd[q][:], pt[:, base:base + PDB])

                dma_p(0, split=True)
                dma_p(1)
                dma_pd(0)
                for q in range(2, NB):
                    dma_p(q)
                    dma_pd(q - 1)
                dma_pd(NB - 1)
                return cb, pd

            def quant_part1():
                am_p = io.tile([128, 1], dt.float32, tag="amp")
                nc.vector.tensor_reduce(
                    am_p[:], xs_t[:], axis=mybir.AxisListType.X,
                    op=AluOp.max, apply_absolute_value=True,
                )
                am = io.tile([128, 1], dt.float32, tag="am")
                if mode == "fullnog":
                    # perf probe: skip the cross-partition reduce (WRONG
                    # numerics; lower bound for any all-reduce scheme)
                    nc.vector.tensor_copy(am[:], am_p[:])
                elif mode == "fullgp":
                    # old path: GPSIMD daisy-chain all-reduce.  Its exclusive
                    # lock on the shared SBUF port pair blocks the DVE's
                    # 2-port plane ops for its whole duration (~3us) -- kept
                    # only as an A/B probe.
                    nc.gpsimd.partition_all_reduce(
                        am[:], am_p[:], channels=128,
                        reduce_op=bass_isa.ReduceOp.absmax,
                    )
                else:
                    # lock-free cross-partition max: PE-transpose am_p to
                    # [1,128], DVE max-reduce to [1,1], broadcast back to
                    # [128,1] with a K=1 ones-row matmul, ACT-copy to SBUF.
                    tp = ps2.tile([1, 128], dt.float32)
                    nc.tensor.transpose(tp[0:1, :], am_p[:], idn_t[:])
                    am1 = io.tile([1, 1], dt.float32, tag="am1")
                    nc.vector.tensor_reduce(
                        am1[0:1, :], tp[0:1, :], axis=mybir.AxisListType.X,
                        op=AluOp.max,
                    )
                    ambc = ps2.tile([128, 1], dt.float32)
                    nc.tensor.matmul(
                        ambc[:], ones_r[0:1, :], am1[0:1, :],
                        start=True, stop=True,
                    )
                    nc.scalar.copy(am[:], ambc[:])
                return am

            def quant_part2(am):
                # act_scale/127, r, g512; xq+scale on ACT (exact: subtract
                # then multiply); rank-1 row values for the correction.
                asc = io.tile([128, 1], dt.float32, tag="asc")
                nc.vector.tensor_scalar(
                    out=asc[:], in0=am[:], scalar1=1e-5,
                    scalar2=1.0 / 127.0, op0=AluOp.max, op1=AluOp.mult,
                )
                r = io.tile([128, 1], dt.float32, tag="r")
                nc.vector.reciprocal(r[:], asc[:])
                g512 = io.tile([128, 1], dt.float32, tag="g512")
                nc.vector.tensor_scalar(
                    out=g512[:], in0=asc[:], scalar1=512.0,
                    scalar2=ws_b[:], op0=AluOp.mult, op1=AluOp.mult,
                )
                xq_f = io.tile([128, 512], dt.float32, tag="xqf")
                nc.scalar.activation(
                    xq_f[:], xs_t[:], ActFn.Identity,
                    bias=magic_t[:], scale=r[:],
                )
                xq_i = io.tile([128, 512], dt.float32, tag="xqi")
                nc.scalar.activation(
                    xq_i[:], xq_f[:], ActFn.Identity, bias=nmagic_t[:],
                )
                xqs = io.tile([128, 512], dt.bfloat16, tag="xqs")
                nc.scalar.activation(
                    xqs[:], xq_i[:], ActFn.Copy, scale=g512[:],
                )
                t_pb = io.tile([128, 8], dt.float32, tag="tpb")
                nc.vector.tensor_reduce(
                    t_pb[:],
                    xqs[:].rearrange("p (t b) -> p b t", t=4 * NJT, b=8),
                    axis=mybir.AxisListType.X, op=AluOp.add,
                )
                exl = io.tile([2, 8], dt.float32, tag="exl")
                nc.vector.memset(exl[:], 1.0)
                rowps = ps2.tile([1, 8], dt.float32)
                nc.tensor.matmul(
                    rowps[0:1, :], ones_t[:], t_pb[:], start=True, stop=True,
                )
                nc.scalar.mul(exl[0:1, :], rowps[0:1, :], -1.0 / 512.0)
                return xqs, exl

            if mode == "dmaraw":
                zt = io.tile([8, OS], dt.float32)
                nc.vector.memset(zt[:], 0.0)
                for _rep in range(repeat):
                    issue_dmas()
                nc.sync.dma_start(out[:, :], zt[:])
                repeat = 0

            if mode == "planesonly":
                zt = io.tile([8, OS], dt.float32)
                nc.vector.memset(zt[:], 0.0)
                cb0 = io.tile([128, PKB], dt.uint8)
                nc.sync.dma_start(cb0[:], pt[:, 0:PKB])
                for _rep in range(repeat):
                    for q in range(NB):
                        for k in range(4):
                            plane_full(nc, plpool, cb0, k)
                nc.sync.dma_start(out[:, :], zt[:])
                repeat = 0

            if mode in ("quantonly", "quantnog"):
                zt = io.tile([8, OS], dt.float32)
                nc.vector.memset(zt[:], 0.0)
                for _rep in range(repeat):
                    if mode == "quantonly":
                        am = quant_part1()
                    else:
                        am_p = io.tile([128, 1], dt.float32, tag="amp")
                        nc.vector.tensor_reduce(
                            am_p[:], xs_t[:], axis=mybir.AxisListType.X,
                            op=AluOp.max, apply_absolute_value=True,
                        )
                        am = io.tile([128, 1], dt.float32, tag="am")
                        nc.vector.tensor_copy(am[:], am_p[:])
                    quant_part2(am)
                nc.sync.dma_start(out[:, :], zt[:])
                repeat = 0

            xqs = exl = None
            for _rep in range(repeat):
                do_quant = (_rep == 0) or (mode != "fullnq")

                if mode == "peonly":
                    pks0 = {}
                    for k in range(4):
                        t = io.tile(
                            [128, JPB * kwidth(k)], dt.int32, tag=f"rpk{k}",
                            name=f"rpk{k}",
                        )
                        nc.vector.memset(t[:], 0.0)
                        pks0[k] = t
                    pd_res = io.tile([128, PDB], dt.uint8, tag="rpd")
                    nc.vector.memset(pd_res[:].bitcast(dt.int32), 0.0)
                    cb = [None] * NB
                    pd = [pd_res] * NB
                else:
                    cb, pd = issue_dmas()

                if do_quant:
                    am = quant_part1()

                # block-0 plane extraction, jt0 half first (covers the
                # GPSIMD all-reduce latency; DVE never idles)
                if mode != "peonly":
                    pks0 = {
                        k: plpool.tile(
                            [128, JPB * kwidth(k)], dt.int32, tag=f"pk{k}",
                            name=f"pk{k}",
                        )
                        for k in range(4)
                    }
                    for k in range(4):
                        w = kwidth(k)
                        h0 = cb[0][:, 0:4 * w].bitcast(dt.int32)
                        plane_op(nc, pks0[k][:, 0:w], h0, k)

                if do_quant:
                    xqs, exl = quant_part2(am)

                if mode != "peonly":
                    for k in range(4):
                        w = kwidth(k)
                        h1 = cb[0][:, OW:OW + 4 * w].bitcast(dt.int32)
                        plane_op(nc, pks0[k][:, w:2 * w], h1, k)

                # ---------- main loop: stream, unpack, matmul ----------
                acc = ps.tile([128, 1024], dt.float32)  # chunk cc at cc*512

                def mm_block(q, k, pk, pd_t):
                    pk8 = pk[:].bitcast(dt.float8e4)
                    pd8 = pd_t[:].bitcast(dt.float8e4)
                    narrowed = k >= 4 - PD6K
                    SW = OWN if narrowed else OW
                    for sub in range(JPB):
                        jt = JPB * q + sub
                        lhsT = xqs[:, (jt * 4 + k) * 8:(jt * 4 + k + 1) * 8]
                        first = (q == 0 and k == 0 and sub == 0)
                        last = (q == NB - 1 and k == 3 and sub == JPB - 1)
                        for cc in range(2):
                            for g in range(NG):
                                m = 2 * g + cc
                                if m < NPK - 1 or (m == NPK - 1
                                                   and not narrowed):
                                    rhs = pk8[:, sub * SW + m * CH:
                                              sub * SW + (m + 1) * CH]
                                elif m == NPK - 1:
                                    # chunk 6, predecoded plane
                                    slot = 4 + (k - (4 - PD6K))
                                    rhs = pd8[:, (sub * NPD + slot) * CH:
                                              (sub * NPD + slot + 1) * CH]
                                else:
                                    # chunk 7, predecoded plane k
                                    rhs = pd8[:, (sub * NPD + k) * CH:
                                              (sub * NPD + k + 1) * CH]
                                nc.tensor.matmul(
                                    acc[32 * g:32 * g + 8,
                                        cc * 512:cc * 512 + CH],
                                    lhsT, rhs,
                                    start=first, stop=last,
                                    tile_position=(0, 32 * g),
                                )

                for q in range(NB):
                    if q == 0 or mode == "peonly":
                        pks = pks0
                    else:
                        pks = {
                            k: plane_full(nc, plpool, cb[q], k)
                            for k in range(4)
                        }
                    for k in range(4):
                        mm_block(q, k, pks[k], pd[q])
                    if q == 1:
                        # fold bias + Sx rank-1 rows into PSUM mid-stream so
                        # the post-stream tail is only the copies + DMA
                        for cc in range(2):
                            for g in range(NG):
                                m = 2 * g + cc
                                nc.tensor.matmul(
                                    acc[32 * g:32 * g + 8,
                                        cc * 512:cc * 512 + CH],
                                    exl[:],
                                    ext_t[:, m * CH:(m + 1) * CH],
                                    start=False, stop=False,
                                    tile_position=(0, 32 * g),
                                )

                # ---------- output: PSUM -> SBUF (ACT) -> HBM ----------
                for g in range(NG):
                    ot = opool.tile([8, 2 * CH], dt.float32, tag=f"ot{g % 2}")
                    src = acc[32 * g:32 * g + 8, 0:1024].rearrange(
                        "p (c x) -> p c x", c=2
                    )[:, :, 0:CH]
                    dst = ot[:].rearrange("p (c x) -> p c x", c=2)
                    nc.scalar.copy(dst, src)
                    nc.sync.dma_start(
                        out[:, 2 * g * CH:2 * (g + 1) * CH], ot[:]
                    )

    nc.compile()
    return nc


def host_prep(x, packed_weight, weight_scale, bias):
    x = np.asarray(x, dtype=np.float32)
    packed_weight = np.asarray(packed_weight, dtype=np.int32)
    weight_scale = np.asarray(weight_scale, dtype=np.float32)
    bias = np.asarray(bias, dtype=np.float32)

    # x -> stationary layout [p, (jt k b)]
    xs_np = np.ascontiguousarray(
        x.reshape(B, NJT, 128, 4).transpose(2, 1, 3, 0)
    ).reshape(128, 512)
    ws_np = weight_scale.reshape(1, 1)

    in_maps = []
    for c in range(NCORES):
        sl = slice(c * OS, (c + 1) * OS)
        # low byte only (4 codes); [O_s, J] -> [jt, p, o]
        pu = (packed_weight[sl, :].astype(np.uint32) & 0xFF).astype(np.uint8)
        puT = np.ascontiguousarray(pu.T.reshape(NJT, 128, OS))
        pk_part = puT[:, :, :OW].reshape(NB, JPB, 128, OW)
        c7 = puT[:, :, OW:]  # [jt, p, CH] chunk 7 packed bytes
        c6 = puT[:, :, OWN:OW]  # [jt, p, CH] chunk 6 packed bytes
        pd_planes = np.concatenate(
            [np.stack([(c7 >> (2 * k)) & 3 for k in range(4)], axis=2)]
            + ([np.stack([(c6 >> (2 * k)) & 3
                          for k in range(4 - PD6K, 4)], axis=2)]
               if PD6K else []),
            axis=2,
        ).astype(np.uint8)  # [jt, p, NPD, CH]
        pd_part = pd_planes.reshape(NB, JPB, 128, NPD, CH)
        blocks = np.concatenate([
            pk_part.transpose(2, 0, 1, 3).reshape(128, NB, PKB),
            pd_part.transpose(2, 0, 1, 3, 4).reshape(128, NB, PDB),
        ], axis=2)  # [128, NB, BBLK]
        ptc = np.ascontiguousarray(blocks).reshape(128, NB * BBLK)
        extc = np.empty((2, OS), dtype=np.float32)
        extc[0, :] = 1.0
        extc[1, :] = bias[sl]
        in_maps.append({
            "pt": ptc, "xs": xs_np, "ext": extc, "ws": ws_np,
            "idn": np.eye(128, dtype=np.float32),
        })
    return in_maps


def kernel(x, packed_weight, weight_scale, bias):
    global LAST_RESULTS
    repeat = int(os.environ.get("BITNET_REPEAT", "1"))
    mode = os.environ.get("BITNET_MODE", "full")
    key = (repeat, mode)
    if key not in _cache:
        _cache[key] = _build(repeat, mode=mode)
    nc = _cache[key]

    in_maps = host_prep(x, packed_weight, weight_scale, bias)
    res = run_bass_kernel_spmd(nc, in_maps, list(range(NCORES)))
    LAST_RESULTS = res
    return np.concatenate(
        [np.asarray(res.results[c]["out"]) for c in range(NCORES)], axis=1
    ).reshape(B, O)
